# revision 16
# baseline (speedup 1.0000x reference)
"""Custom LSTM kernel for Trainium2 (8 NeuronCores, SPMD data-parallel over batch).

Per core (batch shard 16): gates in B-layout PSUM [128,512], partitions =
(gate-group, batch) with groups (i,f,o,g) at 32j via PE col-tiling; free = H.
lhsT = [x_t; h]^T chunks [128,16] (tiny LDW), rhs = V=[W;U] chunks streaming.
Bias enters via a full-width M=128 init matmul (sets has_written bank-wide).
sigma applied in PSUM->PSUM; DVE ops mix one PSUM + one base-0 SBUF operand.
h^T via PE transpose feeds both the next step's weights (fp32) and a bf16
output ring dumped to DRAM every 256 steps (keeps total DMA count at 4 so the
kernel-tail Drain stays under the ISA sync-wait budget).
"""

import os
import sys
import numpy as np

for _p in ("/opt/trn_rl_repo", "/root/.axon_site/_ro/trn_rl_repo"):
    if os.path.isdir(_p) and _p not in sys.path:
        sys.path.insert(0, _p)

HS = 512
IN_SZ = 256
N_CORES = 8

_CACHE = {}


def _ring(S):
    return 256 if S % 256 == 0 else S


def _build(S, B_PER_CORE):
    import concourse.bass as bass
    import concourse.mybir as mybir
    import concourse.tile as tile

    fp32 = mybir.dt.float32
    bf16 = mybir.dt.bfloat16
    nc = bass.Bass()

    NB = B_PER_CORE
    NKX, NKH = 2, 4
    NKV = NKX + NKH
    RING = _ring(S)
    ND = S // RING
    XC, VC = NKX * S * NB, NKV * 2048
    TOT = XC + VC + 768

    allin_d = nc.dram_tensor("allin", [128, TOT], fp32, kind="ExternalInput")
    hTseq_d = nc.dram_tensor("hTseq", [ND, 128, RING * 4 * NB], bf16,
                             kind="ExternalOutput")
    clast_d = nc.dram_tensor("clast", [NB, HS], fp32, kind="ExternalOutput")

    SIG = mybir.ActivationFunctionType.Sigmoid
    TANH = mybir.ActivationFunctionType.Tanh

    with tile.TileContext(nc) as tc:
        with (
            tc.tile_pool(name="const", bufs=1) as cpool,
            tc.tile_pool(name="state", bufs=1) as spool,
            tc.tile_pool(name="gates", bufs=2) as gpool,
            tc.tile_pool(name="work", bufs=2) as wpool,
            tc.tile_pool(name="hT", bufs=2) as hpool,
            tc.tile_pool(name="psg", bufs=2, space="PSUM") as psg_pool,
            tc.tile_pool(name="pst", bufs=2, space="PSUM") as pst_pool,
        ):
            allin_sb = cpool.tile([128, TOT], fp32)
            nc.sync.dma_start(allin_sb[:], allin_d[:, :])
            xT_sb = allin_sb[:, 0:XC].rearrange("p (c n) -> p c n", c=NKX)
            V_sb = allin_sb[:, XC:XC + VC].rearrange("p (c n) -> p c n", c=NKV)
            B4_sb = allin_sb[:, XC + VC:XC + VC + 128]
            bias4_sb = allin_sb[:, XC + VC + 128:XC + VC + 640]
            ident_sb = allin_sb[:, XC + VC + 640:XC + VC + 768]

            # single warmup matmul consumes the load DMA's semaphore on PE
            ps_w = pst_pool.tile([128, NB], fp32, tag="warm")
            nc.tensor.matmul(ps_w[0:NB, :], allin_sb[:, 0:NB],
                             allin_sb[:, NB:2 * NB], start=True, stop=True,
                             skip_group_check=True)

            c_sb = spool.tile([NB, HS], fp32, tag="c")
            ring_sb = spool.tile([128, RING * 4 * NB], bf16, tag="ring")

            hT_prev = None
            for t in range(S):
                ps_g = psg_pool.tile([128, HS], fp32, tag="psg")
                nc.tensor.matmul(ps_g[:, :], B4_sb, bias4_sb,
                                 start=True, stop=False, skip_group_check=True)
                n_k = NKX + (NKH if t > 0 else 0)
                ki = 0
                for kc in range(NKX):
                    lhsT = xT_sb[:, kc, t * NB:(t + 1) * NB]
                    ki += 1
                    for j in range(4):
                        nc.tensor.matmul(
                            ps_g[32 * j:32 * j + NB, :], lhsT,
                            V_sb[:, kc, j * HS:(j + 1) * HS],
                            start=False, stop=(ki == n_k and j == 3),
                            tile_position=(0, 32 * j), skip_group_check=True)
                if t > 0:
                    for kc in range(NKH):
                        lhsT = hT_prev[:, kc, :]
                        ki += 1
                        for j in range(4):
                            nc.tensor.matmul(
                                ps_g[32 * j:32 * j + NB, :], lhsT,
                                V_sb[:, NKX + kc, j * HS:(j + 1) * HS],
                                start=False, stop=(ki == n_k and j == 3),
                                tile_position=(0, 32 * j), skip_group_check=True)

                ps_a = psg_pool.tile([128, HS], fp32, tag="psact")
                gates = gpool.tile([128, HS], fp32, tag="gates")
                nc.scalar.activation(ps_a[0:80, :], ps_g[0:80, :], SIG, scale=2.0)
                nc.scalar.activation(gates[96:112, :], ps_g[96:112, :], TANH,
                                     scale=2.0)

                ig = wpool.tile([NB, HS], fp32, tag="ig")
                cnew = wpool.tile([NB, HS], fp32, tag="cnew")
                tanh_c = wpool.tile([NB, HS], fp32, tag="thc")
                h_sb = wpool.tile([NB, HS], fp32, tag="h")

                nc.vector.tensor_mul(ig[:], ps_a[0:NB, :], gates[96:96 + NB, :])
                if t > 0:
                    fc = wpool.tile([NB, HS], fp32, tag="fc")
                    nc.vector.tensor_mul(fc[:], ps_a[32:32 + NB, :], c_sb[:])
                    nc.vector.tensor_add(cnew[:], fc[:], ig[:])
                else:
                    nc.vector.tensor_copy(cnew[:], ig[:])
                nc.scalar.activation(tanh_c[:], cnew[:], TANH, scale=1.0)
                nc.vector.tensor_mul(h_sb[:], ps_a[64:64 + NB, :], tanh_c[:])
                nc.vector.tensor_scalar_mul(c_sb[:], cnew[:], 0.5)

                # h^T: PE transposes -> fp32 hT (next step) + bf16 ring (output)
                ps_t = pst_pool.tile([128, NKH * NB], fp32, tag="pst")
                for kc in range(NKH):
                    nc.tensor.transpose(
                        ps_t[:, kc * NB:(kc + 1) * NB],
                        h_sb[:, kc * 128:(kc + 1) * 128],
                        ident_sb[0:NB, 0:NB])
                r = t % RING
                nc.scalar.copy(
                    ring_sb[:, r * (NKH * NB):(r + 1) * (NKH * NB)], ps_t[:, :])
                if t < S - 1:
                    hT = hpool.tile([128, NKH, NB], fp32, tag="hT")
                    nc.vector.tensor_copy(hT[:, :, :], ps_t[:, :])
                    hT_prev = hT
                if r == RING - 1:
                    nc.sync.dma_start(hTseq_d[t // RING], ring_sb[:, :])

            nc.sync.dma_start(clast_d[:, :], c_sb[:])

    _split_waits(nc, mybir)
    return nc


def _split_waits(nc, mybir):
    """Walrus allows ~1 sync wait per instruction; hoist extras onto
    same-engine NOPs inserted just before the offending instruction."""
    ctr = 0
    for func in nc.m.functions:
        for bb in func.blocks:
            insts = bb.instructions
            out = []
            changed = False
            for inst in insts:
                si = inst.sync_info
                waits = list(si.on_wait) if (si and si.on_wait) else []
                if len(waits) > 1:
                    for w in waits[:-1]:
                        ctr += 1
                        nop = mybir.InstNoOp(
                            name=f"I-waitnop-{ctr}", ins=[], outs=[])
                        nop.engine = inst.engine
                        nop.sync_info = mybir.SyncInfo(on_wait=[w],
                                                       on_update=[])
                        out.append(nop)
                    si.on_wait = waits[-1:]
                    changed = True
                out.append(inst)
            if changed:
                bb.instructions = out


def _get_program(S, NB):
    key = (S, NB)
    if key not in _CACHE:
        _CACHE[key] = _build(S, NB)
    return _CACHE[key]


def _host_pack(x, W, U, bias, S, NB, k):
    I = IN_SZ
    perm = [0, 1, 3, 2]
    H = HS
    V_full = np.concatenate([W, U], axis=0)
    V = np.empty_like(V_full)
    for j, pg in enumerate(perm):
        V[:, j * H:(j + 1) * H] = V_full[:, pg * H:(pg + 1) * H]
    B4 = np.zeros((128, 128), np.float32)
    for j in range(4):
        B4[j, 32 * j:32 * j + 32] = 1.0
    bias4 = np.zeros((128, H), np.float32)
    for j, pg in enumerate(perm):
        bias4[j, :] = bias[pg * H:(pg + 1) * H]
    ident = np.eye(128, dtype=np.float32)

    xk = x[:, k * NB:(k + 1) * NB, :]
    xT = np.ascontiguousarray(xk.transpose(2, 0, 1)).reshape(I, S * NB)
    xT_c = xT.reshape(2, 128, S * NB).transpose(1, 0, 2).reshape(128, -1)
    V_c = V.reshape(6, 128, 2048).transpose(1, 0, 2).reshape(128, -1)
    init = np.concatenate([B4, bias4, ident], axis=1)
    return np.ascontiguousarray(
        np.concatenate([xT_c, V_c, init], axis=1), dtype=np.float32)


def kernel(x, W, U, bias_w, bias_u):
    from concourse import bass_utils

    x = np.asarray(x, dtype=np.float32)
    W = np.asarray(W, dtype=np.float32)
    U = np.asarray(U, dtype=np.float32)
    bias = (np.asarray(bias_w, dtype=np.float32)
            + np.asarray(bias_u, dtype=np.float32))

    S, BS, I = x.shape
    H = U.shape[0]
    NB = BS // N_CORES
    assert I == IN_SZ and H == HS

    nc = _get_program(S, NB)
    in_maps = [{"allin": _host_pack(x, W, U, bias, S, NB, k)}
               for k in range(N_CORES)]
    res = bass_utils.run_bass_kernel_spmd(nc, in_maps, list(range(N_CORES)))

    RING = _ring(S)
    ND = S // RING
    hidden = np.empty((S, BS, H), dtype=np.float32)
    c_last = np.empty((BS, H), dtype=np.float32)
    for k in range(N_CORES):
        arr = np.asarray(res.results[k]["hTseq"]).astype(np.float32)
        arr = arr.reshape(ND, 128, RING, 4, NB)
        hk = np.transpose(arr, (0, 2, 4, 3, 1)).reshape(S, NB, H)
        hidden[:, k * NB:(k + 1) * NB, :] = hk
        c_last[k * NB:(k + 1) * NB, :] = np.asarray(
            res.results[k]["clast"]).astype(np.float32)
    h_last = hidden[-1].copy()
    return hidden, (h_last, c_last)
